# revision 16
# baseline (speedup 1.0000x reference)
"""AffinityPropagate3 Trainium2 kernel, v7.

Per-core (one batch sample): K = softmax(guided, 0); mask = sign(sparse);
x_{t+1} = mask*x0 + (1-mask) * sum_k K_k * shift_k(x_t), 16 steps.

Layout: xa [120 part, 6 row-slots, 642 cols] fp16, double-buffered.
Row slots: 0 = top halo, 1-4 = rows 4p..4p+3, 5 = bottom halo; col 0/641 pad.

v7 structure (trace-driven redesign of v3):
  - All 9 taps emitted as HALF products (output rows 0:2 = A banks,
    2:4 = B banks).  Order closes the B psum banks ~4us before the DVE
    stream ends, so the B drains + top-halo DMA overlap the remaining
    A-half products and the next iteration's first B-half products start
    with zero DVE stall.
  - z tiles double-buffered by iteration parity: a product of iteration
    t never WAR-stalls on iteration t-1's PE pass reading the same z.
  - Taps 4 (both halves) and 8 (A half) run on the otherwise-idle Pool
    engine (gpsimd tensor_tensor).
  - Halo rows move via SBUF->SBUF DMA (sync + gpsimd queues) instead of
    PE shift-matmuls + ACT drains.
  - m0 (mask*x0) is written into PSUM by per-row ACT copies, each woven
    directly after that row's drain; all z matmuls run start=False.
    PE work is the 9 tap passes only (~9.7us at full clock).
  - Per-row-slot drains (4 ACT copies of 640), order rows 3,2 then 1,0.
  - Two keep-warm dummy matmuls into a scratch psum bank at the
    iteration boundary hold the PE p-state up.
"""

import sys

for _p in ("/opt/trn_rl_repo", "/root/.axon_site/_ro/trn_rl_repo"):
    if _p not in sys.path:
        sys.path.insert(0, _p)

import numpy as np

from concourse import bacc, mybir
from concourse import tile
from concourse.bass_utils import run_bass_kernel_spmd


def dedup_ldweights(nc):
    """Drop InstLdweights whose stationary matches the previous one (PE
    weights persist).  An Ldweights carrying sync info becomes a NoOp."""
    for f in nc.m.functions:
        for bb in f.blocks:
            out = []
            seen_key = None
            changed = False
            for ins in bb.instructions:
                if type(ins).__name__ == "InstLdweights":
                    key = str(ins.ins[0])
                    if key == seen_key:
                        si = ins.sync_info
                        if si is not None and (si.on_wait or si.on_update):
                            out.append(
                                mybir.InstNoOp(
                                    name=ins.name + "-ldwn",
                                    engine=ins.engine,
                                    sync_info=si,
                                )
                            )
                        changed = True
                        continue
                    seen_key = key
                out.append(ins)
            if changed:
                bb.instructions[:] = out


B = 8
H, W = 480, 640
P = 120
RPP = 4
WP = W + 2
NJ = RPP + 2
FLAT = RPP * W       # 2560
HFLAT = FLAT // 2    # 1280
PROP_TIME = 16

FP32 = mybir.dt.float32
FP16 = mybir.dt.float16

TAPS = [(ki - 1, kj - 1) for ki in range(3) for kj in range(3)]

# psum chunk lists (matmul out stays within one 2KB bank; all start=False)
A_CHUNKS = [(0, 512), (512, 1024), (1024, 1280)]
B_CHUNKS = [(1280, 1536), (1536, 2048), (2048, 2560)]
FIVE = [(c * 512, (c + 1) * 512) for c in range(5)]

# DVE product order.  half=1 -> output rows 2:4 (B banks), half=0 -> 0:2.
DVE_PART1 = [(3, 1), (5, 1), (1, 1), (0, 1), (2, 1), (3, 0), (5, 0),
             (6, 1), (8, 1), (7, 1)]
DVE_PART2 = [(0, 0), (1, 0), (2, 0), (6, 0), (7, 0)]


def _rows_view(dram_ap):
    return dram_ap.rearrange("(p r) w -> p r w", p=P)


def build_program(compile_=True):
    nc = bacc.Bacc("TRN2", target_bir_lowering=False, debug=False, num_devices=B)

    guided_d = nc.dram_tensor("guided", [9, H, W], FP32, kind="ExternalInput")
    x_d = nc.dram_tensor("x", [H, W], FP32, kind="ExternalInput")
    sparse_d = nc.dram_tensor("sparse_depth", [H, W], FP32, kind="ExternalInput")
    out_d = nc.dram_tensor("out", [H, W], FP32, kind="ExternalOutput")

    ident_d = nc.inline_tensor(np.eye(P, dtype=np.float16), name="ident_const")

    with tile.TileContext(nc) as tc:
        with (
            tc.tile_pool(name="persist", bufs=1) as persist,
            tc.tile_pool(name="work32", bufs=2) as work32,
            tc.tile_pool(name="psum", bufs=1, space="PSUM") as psump,
        ):
            xa = [
                persist.tile([P, NJ, WP], FP16, tag=f"xa{i}", name=f"xa{i}")
                for i in range(2)
            ]
            wk = [
                persist.tile([P, FLAT], FP16, tag=f"wk{k}", name=f"wk{k}")
                for k in range(9)
            ]
            # z double-buffered by iteration parity
            zz = [
                [
                    persist.tile([P, RPP, W], FP16, tag=f"z{par}_{k}",
                                 name=f"z{par}_{k}")
                    for k in range(9)
                ]
                for par in range(2)
            ]
            m0 = persist.tile([P, FLAT], FP16, tag="m0")
            nomask = persist.tile([P, FLAT], FP16, tag="nomask")
            rf16 = persist.tile([P, FLAT], FP16, tag="rf16")
            ident = persist.tile([P, P], FP16, tag="ident")
            stag = persist.tile([P, RPP, W], FP32, tag="stag")

            psum = psump.tile([P, FLAT], FP32, tag="psum")       # banks 0-4
            psum_scr = psump.tile([P, 512], FP32, tag="psum_scr")  # bank 5

            # ---- init ----
            nc.vector.memset(xa[0][:], 0.0)
            nc.gpsimd.memset(xa[1][:], 0.0)
            nc.sync.dma_start(out=ident[:], in_=ident_d[:])

            # ---- x load (cast fp32->fp16 via SWDGE) + initial halos ----
            xd = _rows_view(x_d[:])
            nc.gpsimd.dma_start(out=xa[0][:, 1 : 1 + RPP, 1 : 1 + W], in_=xd)
            nc.gpsimd.dma_start(
                out=xa[0][1:P, 0:1, 1 : 1 + W], in_=xd[0 : P - 1, 3:4, :]
            )
            nc.gpsimd.dma_start(
                out=xa[0][0 : P - 1, 5:6, 1 : 1 + W], in_=xd[1:P, 0:1, :]
            )

            # ---- masks (sparse on scalar queue, ahead of its guided) ----
            sp = work32.tile([P, RPP, W], FP32, tag="g32", name="sp32")
            nc.scalar.dma_start(out=sp[:], in_=_rows_view(sparse_d[:]))
            nc.vector.tensor_scalar(
                out=nomask[:], in0=sp.rearrange("p a b -> p (a b)")[:],
                scalar1=0.0, scalar2=None, op0=mybir.AluOpType.is_equal,
            )
            xv = xa[0][:, 1 : 1 + RPP, 1 : 1 + W]
            m0v = m0.rearrange("p (a b) -> p a b", a=RPP)
            nc.vector.tensor_tensor(
                out=m0v[:], in0=nomask.rearrange("p (a b) -> p a b", a=RPP)[:],
                in1=xv, op=mybir.AluOpType.mult,
            )
            nc.vector.tensor_tensor(
                out=m0v[:], in0=xv, in1=m0v[:], op=mybir.AluOpType.subtract
            )

            # ---- guided loads (3-engine spread) + exp; iteration 0 runs on
            # UNNORMALIZED weights, chasing each channel's DMA inside the
            # load wall: z_k = exp(g_k) * tap_k(x0) accumulates on PE while
            # the denominator accumulates on DVE.  Normalization is applied
            # once at drain time: x1 = m0 + (nomask/den) * psum. ----
            dma_engines = [nc.sync, nc.gpsimd, nc.scalar]
            z0 = zz[0]
            wkv0 = [wk[k].rearrange("p (a b) -> p a b", a=RPP) for k in range(9)]
            den16 = persist.tile([P, FLAT], FP16, tag="den16")
            for k in range(9):
                g32 = work32.tile([P, RPP, W], FP32, tag="g32", name=f"g32_{k}")
                dma_engines[k % 3].dma_start(out=g32[:], in_=_rows_view(guided_d[k]))
                nc.scalar.activation(
                    out=wk[k][:], in_=g32.rearrange("p a b -> p (a b)")[:],
                    func=mybir.ActivationFunctionType.Exp,
                )
                dh, dw = TAPS[k]
                nc.vector.tensor_tensor(
                    out=z0[k][:], in0=wkv0[k][:],
                    in1=xa[0][:, 1 + dh : 1 + dh + RPP, 1 + dw : 1 + dw + W],
                    op=mybir.AluOpType.mult,
                )
                if k == 0:
                    nc.vector.tensor_copy(out=den16[:], in_=wk[0][:])
                else:
                    nc.vector.tensor_tensor(
                        out=den16[:], in0=den16[:], in1=wk[k][:],
                        op=mybir.AluOpType.add,
                    )
                zk = z0[k].rearrange("p a b -> p (a b)")
                for lo, hi in FIVE:
                    nc.tensor.matmul(
                        out=psum[:, lo:hi],
                        lhsT=ident[:],
                        rhs=zk[:, lo:hi],
                        start=(k == 0),
                        stop=(k == 8),
                    )

            # ---- normalization: rf16 = nomask / sum_k exp ----
            den32 = work32.tile([P, RPP, W], FP32, tag="g32", name="den32")
            den32f = den32.rearrange("p a b -> p (a b)")
            nc.vector.tensor_copy(out=den32f[:], in_=den16[:])
            r32 = work32.tile([P, RPP, W], FP32, tag="g32", name="r32")
            r32f = r32.rearrange("p a b -> p (a b)")
            nc.vector.reciprocal_approx_fast(out=r32f[:], in_=den32f[:])
            nc.vector.tensor_tensor(
                out=rf16[:], in0=r32f[:], in1=nomask[:], op=mybir.AluOpType.mult
            )

            # helpers -------------------------------------------------------
            def tap_view(xsrc, k, half):
                dh, dw = TAPS[k]
                r0 = 1 + dh + 2 * half
                c0 = 1 + dw
                return xsrc[:, r0 : r0 + 2, c0 : c0 + W]

            def prod(eng, par, xsrc, k, half):
                wv = wk[k].rearrange("p (a b) -> p a b", a=RPP)
                eng.tensor_tensor(
                    out=zz[par][k][:, 2 * half : 2 * half + 2],
                    in0=wv[:, 2 * half : 2 * half + 2],
                    in1=tap_view(xsrc, k, half),
                    op=mybir.AluOpType.mult,
                )

            def scale_wk(eng, k, half=None):
                lo = 0 if half in (None, 0) else HFLAT
                hi = FLAT if half in (None, 1) else HFLAT
                eng.tensor_tensor(
                    out=wk[k][:, lo:hi], in0=wk[k][:, lo:hi],
                    in1=rf16[:, lo:hi], op=mybir.AluOpType.mult,
                )

            def pe_pass(par, k, half, stop=False):
                zf = zz[par][k].rearrange("p a b -> p (a b)")
                for lo, hi in (B_CHUNKS if half else A_CHUNKS):
                    nc.tensor.matmul(
                        out=psum[:, lo:hi], lhsT=ident[:], rhs=zf[:, lo:hi],
                        start=False, stop=stop,
                    )

            def pe_dummy(n):
                # keep-warm matmuls into the scratch psum bank (never read);
                # no input deps, so they fire as soon as PE is free.
                for _ in range(n):
                    nc.tensor.matmul(
                        out=psum_scr[:], lhsT=ident[:], rhs=m0[:, 0:512],
                        start=True, stop=True,
                    )

            pv = psum.rearrange("p (a b) -> p a b", a=RPP)
            rfv = rf16.rearrange("p (a b) -> p a b", a=RPP)

            def drain_slot(r, xaw):
                # psum row r -> xaw slot 1+r (fp32 -> fp16)
                nc.scalar.copy(
                    out=xaw[:, 1 + r : 2 + r, 1 : 1 + W], in_=pv[:, r : r + 1]
                )

            def preload_m0_row(r):
                # m0 row r -> psum row r (ACT write; woven right after that
                # row's drain so following z matmuls accumulate onto it)
                nc.scalar.copy(out=pv[:, r : r + 1], in_=m0v[:, r : r + 1])

            def drain_slot_t0(r, xaw):
                # iteration 0 normalizes at drain: x1 = m0 + rf16 * psum_raw
                nc.vector.tensor_tensor(
                    out=stag[:, r : r + 1], in0=rfv[:, r : r + 1],
                    in1=pv[:, r : r + 1], op=mybir.AluOpType.mult,
                )
                nc.vector.tensor_tensor(
                    out=xaw[:, 1 + r : 2 + r, 1 : 1 + W], in0=stag[:, r : r + 1],
                    in1=m0v[:, r : r + 1], op=mybir.AluOpType.add,
                )

            def halo_top(xaw):
                # slot0[p] <- slot4[p-1]  (row 4p-1), after slot4 drain
                nc.sync.dma_start(
                    out=xaw[1:P, 0:1, 1 : 1 + W], in_=xaw[0 : P - 1, 4:5, 1 : 1 + W]
                )

            def halo_bot(xaw):
                # slot5[p] <- slot1[p+1]  (row 4p+4), after slot1 drain
                nc.gpsimd.dma_start(
                    out=xaw[0 : P - 1, 5:6, 1 : 1 + W], in_=xaw[1:P, 1:2, 1 : 1 + W]
                )

            odv = _rows_view(out_d[:])

            # ---- 16 propagation iterations (iteration 0 = the raw pass
            # emitted above; only its drain + tail are emitted here) ----
            scaled = set()

            def maybe_scale(eng, k):
                if k not in scaled:
                    scaled.add(k)
                    scale_wk(eng, k)

            for t in range(PROP_TIME):
                par = t % 2
                xar = xa[t % 2]
                xaw = xa[1 - t % 2]
                last = t == PROP_TIME - 1
                first = t == 1

                if t > 0:
                    # Pool stream: tap 4 both halves; tap 8 A half except at
                    # t==1 (where Pool's time goes to the wk4 scales).
                    if first:
                        scale_wk(nc.gpsimd, 4, half=1)
                    prod(nc.gpsimd, par, xar, 4, 1)
                    if first:
                        scale_wk(nc.gpsimd, 4, half=0)
                    prod(nc.gpsimd, par, xar, 4, 0)
                    if not first:
                        prod(nc.gpsimd, par, xar, 8, 0)

                    # part 1: B halves (+ two A fillers), closing B banks.
                    for k, half in DVE_PART1:
                        if first:
                            maybe_scale(nc.vector, k)
                        prod(nc.vector, par, xar, k, half)
                        if (k, half) == (0, 1):
                            pe_pass(par, 3, 1)
                            pe_pass(par, 5, 1)
                            pe_pass(par, 1, 1)
                            pe_pass(par, 0, 1)
                            pe_pass(par, 4, 1)   # pool product
                        elif (k, half) == (3, 0):
                            pe_pass(par, 2, 1)
                            pe_pass(par, 3, 0)
                        elif (k, half) == (6, 1):
                            pe_pass(par, 5, 0)
                            pe_pass(par, 6, 1)
                        elif (k, half) == (8, 1):
                            pe_pass(par, 8, 1)
                        elif (k, half) == (7, 1):
                            pe_pass(par, 7, 1, stop=True)  # closes B banks

                    # B drains, each followed by its m0 preload; top halo.
                    if not last:
                        drain_slot(3, xaw)
                        preload_m0_row(3)
                        drain_slot(2, xaw)
                        preload_m0_row(2)
                        halo_top(xaw)
                    else:
                        nc.scalar.copy(out=stag[:, 2:4], in_=pv[:, 2:4])
                        nc.sync.dma_start(out=odv[:, 2:4], in_=stag[:, 2:4])

                    # part 2: A halves, closing A banks.
                    for k, half in DVE_PART2:
                        if first:
                            maybe_scale(nc.vector, k)
                        prod(nc.vector, par, xar, k, half)
                        if first and (k, half) == (2, 0):
                            prod(nc.vector, par, xar, 8, 0)
                        if (k, half) == (1, 0):
                            pe_pass(par, 4, 0)   # pool product
                            pe_pass(par, 0, 0)
                            pe_pass(par, 1, 0)
                        elif (k, half) == (2, 0):
                            pe_pass(par, 2, 0)
                        elif (k, half) == (6, 0):
                            pe_pass(par, 8, 0)   # pool product
                            pe_pass(par, 6, 0)
                        elif (k, half) == (7, 0):
                            pe_pass(par, 7, 0, stop=True)  # closes A banks
                            if not last:
                                pe_dummy(2)

                    if not last:
                        drain_slot(1, xaw)
                        preload_m0_row(1)
                        drain_slot(0, xaw)
                        preload_m0_row(0)
                        halo_bot(xaw)
                    else:
                        nc.scalar.copy(out=stag[:, 0:2], in_=pv[:, 0:2])
                        nc.sync.dma_start(out=odv[:, 0:2], in_=stag[:, 0:2])
                else:
                    # iteration 0 tail: DVE-normalized drains, each row's m0
                    # preload woven behind it, + halos.
                    drain_slot_t0(3, xaw)
                    preload_m0_row(3)
                    drain_slot_t0(2, xaw)
                    preload_m0_row(2)
                    halo_top(xaw)
                    drain_slot_t0(1, xaw)
                    preload_m0_row(1)
                    drain_slot_t0(0, xaw)
                    preload_m0_row(0)
                    halo_bot(xaw)

    dedup_ldweights(nc)
    if compile_:
        nc.compile()
    return nc


_CACHED_NC = None


def _get_nc():
    global _CACHED_NC
    if _CACHED_NC is None:
        _CACHED_NC = build_program()
    return _CACHED_NC


def kernel(guided, x, sparse_depth, _trace=False, _trace_kwargs=None):
    guided = np.ascontiguousarray(guided, dtype=np.float32)
    x = np.ascontiguousarray(x, dtype=np.float32)
    sparse_depth = np.ascontiguousarray(sparse_depth, dtype=np.float32)
    assert guided.shape == (B, 9, H, W)

    nc = _get_nc()
    in_maps = [
        {
            "guided": guided[b],
            "x": x[b, 0],
            "sparse_depth": sparse_depth[b, 0],
        }
        for b in range(B)
    ]
    res = run_bass_kernel_spmd(
        nc, in_maps, list(range(B)), trace=_trace, **(_trace_kwargs or {})
    )
    out = np.stack([res.results[b]["out"] for b in range(B)])[:, None]
    if _trace:
        return out.astype(np.float32), res
    return out.astype(np.float32)


# revision 18
# speedup vs baseline: 1.1139x; 1.1139x over previous
"""AffinityPropagate3 Trainium2 kernel, v7.

Per-core (one batch sample): K = softmax(guided, 0); mask = sign(sparse);
x_{t+1} = mask*x0 + (1-mask) * sum_k K_k * shift_k(x_t), 16 steps.

Layout: xa [120 part, 6 row-slots, 642 cols] fp16, double-buffered.
Row slots: 0 = top halo, 1-4 = rows 4p..4p+3, 5 = bottom halo; col 0/641 pad.

v7 structure (trace-driven redesign of v3):
  - All 9 taps emitted as HALF products (output rows 0:2 = A banks,
    2:4 = B banks).  Order closes the B psum banks ~4us before the DVE
    stream ends, so the B drains + top-halo DMA overlap the remaining
    A-half products and the next iteration's first B-half products start
    with zero DVE stall.
  - z tiles double-buffered by iteration parity: a product of iteration
    t never WAR-stalls on iteration t-1's PE pass reading the same z.
  - Taps 4 (both halves) and 8 (A half) run on the otherwise-idle Pool
    engine (gpsimd tensor_tensor).
  - Halo rows move via SBUF->SBUF DMA (sync + gpsimd queues) instead of
    PE shift-matmuls + ACT drains.
  - m0 (mask*x0) is written into PSUM by per-row ACT copies, each woven
    directly after that row's drain; all z matmuls run start=False.
    PE work is the 9 tap passes only (~9.7us at full clock).
  - Per-row-slot drains (4 ACT copies of 640), order rows 3,2 then 1,0.
  - Two keep-warm dummy matmuls into a scratch psum bank at the
    iteration boundary hold the PE p-state up.
"""

import sys

for _p in ("/opt/trn_rl_repo", "/root/.axon_site/_ro/trn_rl_repo"):
    if _p not in sys.path:
        sys.path.insert(0, _p)

import numpy as np

from concourse import bacc, mybir
from concourse import tile
from concourse.bass_utils import run_bass_kernel_spmd


def dedup_ldweights(nc):
    """Drop InstLdweights whose stationary matches the previous one (PE
    weights persist).  An Ldweights carrying sync info becomes a NoOp."""
    for f in nc.m.functions:
        for bb in f.blocks:
            out = []
            seen_key = None
            changed = False
            for ins in bb.instructions:
                if type(ins).__name__ == "InstLdweights":
                    key = str(ins.ins[0])
                    if key == seen_key:
                        si = ins.sync_info
                        if si is not None and (si.on_wait or si.on_update):
                            out.append(
                                mybir.InstNoOp(
                                    name=ins.name + "-ldwn",
                                    engine=ins.engine,
                                    sync_info=si,
                                )
                            )
                        changed = True
                        continue
                    seen_key = key
                out.append(ins)
            if changed:
                bb.instructions[:] = out


B = 8
H, W = 480, 640
P = 120
RPP = 4
WP = W + 2
NJ = RPP + 2
FLAT = RPP * W       # 2560
HFLAT = FLAT // 2    # 1280
PROP_TIME = 16

FP32 = mybir.dt.float32
FP16 = mybir.dt.float16

TAPS = [(ki - 1, kj - 1) for ki in range(3) for kj in range(3)]

# psum chunk lists (matmul out stays within one 2KB bank; all start=False)
A_CHUNKS = [(0, 512), (512, 1024), (1024, 1280)]
B_CHUNKS = [(1280, 1536), (1536, 2048), (2048, 2560)]
FIVE = [(c * 512, (c + 1) * 512) for c in range(5)]

# DVE product order.  half=1 -> output rows 2:4 (B banks), half=0 -> 0:2.
DVE_PART1 = [(3, 1), (5, 1), (4, 1), (1, 1), (0, 1), (2, 1), (3, 0),
             (6, 1), (8, 1), (7, 1)]
DVE_PART2 = [(5, 0), (4, 0), (1, 0), (0, 0), (2, 0), (6, 0), (8, 0), (7, 0)]


def _rows_view(dram_ap):
    return dram_ap.rearrange("(p r) w -> p r w", p=P)


def build_program(compile_=True):
    nc = bacc.Bacc("TRN2", target_bir_lowering=False, debug=False, num_devices=B)

    guided_d = nc.dram_tensor("guided", [9, H, W], FP32, kind="ExternalInput")
    x_d = nc.dram_tensor("x", [H, W], FP32, kind="ExternalInput")
    sparse_d = nc.dram_tensor("sparse_depth", [H, W], FP32, kind="ExternalInput")
    out_d = nc.dram_tensor("out", [H, W], FP32, kind="ExternalOutput")

    ident_d = nc.inline_tensor(np.eye(P, dtype=np.float16), name="ident_const")

    with tile.TileContext(nc) as tc:
        with (
            tc.tile_pool(name="persist", bufs=1) as persist,
            tc.tile_pool(name="work32", bufs=2) as work32,
            tc.tile_pool(name="psum", bufs=1, space="PSUM") as psump,
        ):
            xa = [
                persist.tile([P, NJ, WP], FP16, tag=f"xa{i}", name=f"xa{i}")
                for i in range(2)
            ]
            wk = [
                persist.tile([P, FLAT], FP16, tag=f"wk{k}", name=f"wk{k}")
                for k in range(9)
            ]
            # z double-buffered by iteration parity
            zz = [
                [
                    persist.tile([P, RPP, W], FP16, tag=f"z{par}_{k}",
                                 name=f"z{par}_{k}")
                    for k in range(9)
                ]
                for par in range(2)
            ]
            m0 = persist.tile([P, FLAT], FP16, tag="m0")
            nomask = persist.tile([P, FLAT], FP16, tag="nomask")
            rf16 = persist.tile([P, FLAT], FP16, tag="rf16")
            ident = persist.tile([P, P], FP16, tag="ident")
            stag = persist.tile([P, RPP, W], FP32, tag="stag")

            psum = psump.tile([P, FLAT], FP32, tag="psum")       # banks 0-4
            psum_scr = psump.tile([P, 512], FP32, tag="psum_scr")  # bank 5

            # ---- init ----
            nc.vector.memset(xa[0][:], 0.0)
            nc.gpsimd.memset(xa[1][:], 0.0)
            nc.sync.dma_start(out=ident[:], in_=ident_d[:])

            # ---- x load (cast fp32->fp16 via SWDGE) + initial halos ----
            xd = _rows_view(x_d[:])
            nc.gpsimd.dma_start(out=xa[0][:, 1 : 1 + RPP, 1 : 1 + W], in_=xd)
            nc.gpsimd.dma_start(
                out=xa[0][1:P, 0:1, 1 : 1 + W], in_=xd[0 : P - 1, 3:4, :]
            )
            nc.gpsimd.dma_start(
                out=xa[0][0 : P - 1, 5:6, 1 : 1 + W], in_=xd[1:P, 0:1, :]
            )

            # ---- masks (sparse on scalar queue, ahead of its guided) ----
            sp = work32.tile([P, RPP, W], FP32, tag="g32", name="sp32")
            nc.scalar.dma_start(out=sp[:], in_=_rows_view(sparse_d[:]))
            nc.vector.tensor_scalar(
                out=nomask[:], in0=sp.rearrange("p a b -> p (a b)")[:],
                scalar1=0.0, scalar2=None, op0=mybir.AluOpType.is_equal,
            )
            xv = xa[0][:, 1 : 1 + RPP, 1 : 1 + W]
            m0v = m0.rearrange("p (a b) -> p a b", a=RPP)
            nc.vector.tensor_tensor(
                out=m0v[:], in0=nomask.rearrange("p (a b) -> p a b", a=RPP)[:],
                in1=xv, op=mybir.AluOpType.mult,
            )
            nc.vector.tensor_tensor(
                out=m0v[:], in0=xv, in1=m0v[:], op=mybir.AluOpType.subtract
            )

            # ---- guided loads (3-engine spread) + exp; iteration 0 runs on
            # UNNORMALIZED weights, chasing each channel's DMA inside the
            # load wall: z_k = exp(g_k) * tap_k(x0) accumulates on PE while
            # the denominator accumulates on DVE.  Normalization is applied
            # once at drain time: x1 = m0 + (nomask/den) * psum. ----
            dma_engines = [nc.sync, nc.gpsimd, nc.scalar]
            z0 = zz[0]
            wkv0 = [wk[k].rearrange("p (a b) -> p a b", a=RPP) for k in range(9)]
            den16 = persist.tile([P, FLAT], FP16, tag="den16")
            for k in range(9):
                g32 = work32.tile([P, RPP, W], FP32, tag="g32", name=f"g32_{k}")
                dma_engines[k % 3].dma_start(out=g32[:], in_=_rows_view(guided_d[k]))
                nc.scalar.activation(
                    out=wk[k][:], in_=g32.rearrange("p a b -> p (a b)")[:],
                    func=mybir.ActivationFunctionType.Exp,
                )
                dh, dw = TAPS[k]
                nc.vector.tensor_tensor(
                    out=z0[k][:], in0=wkv0[k][:],
                    in1=xa[0][:, 1 + dh : 1 + dh + RPP, 1 + dw : 1 + dw + W],
                    op=mybir.AluOpType.mult,
                )
                if k == 0:
                    nc.vector.tensor_copy(out=den16[:], in_=wk[0][:])
                else:
                    nc.vector.tensor_tensor(
                        out=den16[:], in0=den16[:], in1=wk[k][:],
                        op=mybir.AluOpType.add,
                    )
                zk = z0[k].rearrange("p a b -> p (a b)")
                for lo, hi in FIVE:
                    nc.tensor.matmul(
                        out=psum[:, lo:hi],
                        lhsT=ident[:],
                        rhs=zk[:, lo:hi],
                        start=(k == 0),
                        stop=(k == 8),
                    )

            # ---- normalization: rf16 = nomask / sum_k exp ----
            den32 = work32.tile([P, RPP, W], FP32, tag="g32", name="den32")
            den32f = den32.rearrange("p a b -> p (a b)")
            nc.vector.tensor_copy(out=den32f[:], in_=den16[:])
            r32 = work32.tile([P, RPP, W], FP32, tag="g32", name="r32")
            r32f = r32.rearrange("p a b -> p (a b)")
            nc.vector.reciprocal_approx_fast(out=r32f[:], in_=den32f[:])
            nc.vector.tensor_tensor(
                out=rf16[:], in0=r32f[:], in1=nomask[:], op=mybir.AluOpType.mult
            )

            # helpers -------------------------------------------------------
            def tap_view(xsrc, k, half):
                dh, dw = TAPS[k]
                r0 = 1 + dh + 2 * half
                c0 = 1 + dw
                return xsrc[:, r0 : r0 + 2, c0 : c0 + W]

            def prod(eng, par, xsrc, k, half):
                wv = wk[k].rearrange("p (a b) -> p a b", a=RPP)
                eng.tensor_tensor(
                    out=zz[par][k][:, 2 * half : 2 * half + 2],
                    in0=wv[:, 2 * half : 2 * half + 2],
                    in1=tap_view(xsrc, k, half),
                    op=mybir.AluOpType.mult,
                )

            def scale_wk(eng, k, half=None):
                lo = 0 if half in (None, 0) else HFLAT
                hi = FLAT if half in (None, 1) else HFLAT
                eng.tensor_tensor(
                    out=wk[k][:, lo:hi], in0=wk[k][:, lo:hi],
                    in1=rf16[:, lo:hi], op=mybir.AluOpType.mult,
                )

            def pe_pass(par, k, half, stop=False):
                zf = zz[par][k].rearrange("p a b -> p (a b)")
                for lo, hi in (B_CHUNKS if half else A_CHUNKS):
                    nc.tensor.matmul(
                        out=psum[:, lo:hi], lhsT=ident[:], rhs=zf[:, lo:hi],
                        start=False, stop=stop,
                    )

            def pe_dummy(n):
                # keep-warm matmuls into the scratch psum bank (never read);
                # no input deps, so they fire as soon as PE is free.
                for _ in range(n):
                    nc.tensor.matmul(
                        out=psum_scr[:], lhsT=ident[:], rhs=m0[:, 0:512],
                        start=True, stop=True,
                    )

            pv = psum.rearrange("p (a b) -> p a b", a=RPP)
            rfv = rf16.rearrange("p (a b) -> p a b", a=RPP)

            def drain_slot(r, xaw):
                # psum row r -> xaw slot 1+r (fp32 -> fp16)
                nc.scalar.copy(
                    out=xaw[:, 1 + r : 2 + r, 1 : 1 + W], in_=pv[:, r : r + 1]
                )

            def preload_m0_row(r):
                # m0 row r -> psum row r (ACT write; woven right after that
                # row's drain so following z matmuls accumulate onto it)
                nc.scalar.copy(out=pv[:, r : r + 1], in_=m0v[:, r : r + 1])

            def drain_slot_t0(r, xaw):
                # iteration 0 normalizes at drain: x1 = m0 + rf16 * psum_raw
                nc.vector.tensor_tensor(
                    out=stag[:, r : r + 1], in0=rfv[:, r : r + 1],
                    in1=pv[:, r : r + 1], op=mybir.AluOpType.mult,
                )
                nc.vector.tensor_tensor(
                    out=xaw[:, 1 + r : 2 + r, 1 : 1 + W], in0=stag[:, r : r + 1],
                    in1=m0v[:, r : r + 1], op=mybir.AluOpType.add,
                )

            def halo_top(xaw):
                # slot0[p] <- slot4[p-1]  (row 4p-1), after slot4 drain
                nc.sync.dma_start(
                    out=xaw[1:P, 0:1, 1 : 1 + W], in_=xaw[0 : P - 1, 4:5, 1 : 1 + W]
                )

            def halo_bot(xaw):
                # slot5[p] <- slot1[p+1]  (row 4p+4), after slot1 drain
                nc.gpsimd.dma_start(
                    out=xaw[0 : P - 1, 5:6, 1 : 1 + W], in_=xaw[1:P, 1:2, 1 : 1 + W]
                )

            odv = _rows_view(out_d[:])

            # ---- 16 propagation iterations (iteration 0 = the raw pass
            # emitted above; only its drain + tail are emitted here) ----
            scaled = set()

            def maybe_scale(eng, k):
                if k not in scaled:
                    scaled.add(k)
                    scale_wk(eng, k)

            for t in range(PROP_TIME):
                par = t % 2
                xar = xa[t % 2]
                xaw = xa[1 - t % 2]
                last = t == PROP_TIME - 1
                first = t == 1

                if t > 0:
                    # part 1: B halves (+ one A filler), closing B banks.
                    # All products on DVE -- Pool tensor_tensor contends with
                    # DVE for SBUF bandwidth (measured 4x DVE slowdown).
                    for k, half in DVE_PART1:
                        if first:
                            maybe_scale(nc.vector, k)
                        prod(nc.vector, par, xar, k, half)
                        if (k, half) == (1, 1):
                            pe_pass(par, 3, 1)
                            pe_pass(par, 5, 1)
                        elif (k, half) == (0, 1):
                            pe_pass(par, 4, 1)
                            pe_pass(par, 1, 1)
                        elif (k, half) == (2, 1):
                            pe_pass(par, 0, 1)
                        elif (k, half) == (3, 0):
                            pe_pass(par, 2, 1)
                        elif (k, half) == (6, 1):
                            pe_pass(par, 3, 0)
                            pe_pass(par, 6, 1)
                        elif (k, half) == (8, 1):
                            pe_pass(par, 8, 1)
                        elif (k, half) == (7, 1):
                            pe_pass(par, 7, 1, stop=True)  # closes B banks

                    # B drains (top halo right after slot4 lands) + m0 base.
                    if not last:
                        drain_slot(3, xaw)
                        halo_top(xaw)
                        drain_slot(2, xaw)
                        preload_m0_row(3)
                        preload_m0_row(2)
                    else:
                        nc.scalar.copy(out=stag[:, 2:4], in_=pv[:, 2:4])
                        nc.sync.dma_start(out=odv[:, 2:4], in_=stag[:, 2:4])

                    # part 2: A halves, closing A banks.
                    for k, half in DVE_PART2:
                        prod(nc.vector, par, xar, k, half)
                        if (k, half) == (4, 0):
                            pe_pass(par, 5, 0)
                        elif (k, half) == (1, 0):
                            pe_pass(par, 4, 0)
                        elif (k, half) == (0, 0):
                            pe_pass(par, 1, 0)
                        elif (k, half) == (2, 0):
                            pe_pass(par, 0, 0)
                        elif (k, half) == (6, 0):
                            pe_pass(par, 2, 0)
                        elif (k, half) == (8, 0):
                            pe_pass(par, 6, 0)
                        elif (k, half) == (7, 0):
                            pe_pass(par, 8, 0)
                            pe_pass(par, 7, 0, stop=True)  # closes A banks
                            if not last:
                                pe_dummy(2)

                    if not last:
                        drain_slot(1, xaw)
                        drain_slot(0, xaw)
                        halo_bot(xaw)
                        preload_m0_row(1)
                        preload_m0_row(0)
                    else:
                        nc.scalar.copy(out=stag[:, 0:2], in_=pv[:, 0:2])
                        nc.sync.dma_start(out=odv[:, 0:2], in_=stag[:, 0:2])
                else:
                    # iteration 0 tail: DVE-normalized drains + halos + m0.
                    drain_slot_t0(3, xaw)
                    halo_top(xaw)
                    drain_slot_t0(2, xaw)
                    preload_m0_row(3)
                    preload_m0_row(2)
                    drain_slot_t0(1, xaw)
                    drain_slot_t0(0, xaw)
                    halo_bot(xaw)
                    preload_m0_row(1)
                    preload_m0_row(0)

    dedup_ldweights(nc)
    if compile_:
        nc.compile()
    return nc


_CACHED_NC = None


def _get_nc():
    global _CACHED_NC
    if _CACHED_NC is None:
        _CACHED_NC = build_program()
    return _CACHED_NC


def kernel(guided, x, sparse_depth, _trace=False, _trace_kwargs=None):
    guided = np.ascontiguousarray(guided, dtype=np.float32)
    x = np.ascontiguousarray(x, dtype=np.float32)
    sparse_depth = np.ascontiguousarray(sparse_depth, dtype=np.float32)
    assert guided.shape == (B, 9, H, W)

    nc = _get_nc()
    in_maps = [
        {
            "guided": guided[b],
            "x": x[b, 0],
            "sparse_depth": sparse_depth[b, 0],
        }
        for b in range(B)
    ]
    res = run_bass_kernel_spmd(
        nc, in_maps, list(range(B)), trace=_trace, **(_trace_kwargs or {})
    )
    out = np.stack([res.results[b]["out"] for b in range(B)])[:, None]
    if _trace:
        return out.astype(np.float32), res
    return out.astype(np.float32)


# revision 21
# speedup vs baseline: 1.3822x; 1.2409x over previous
"""AffinityPropagate3 Trainium2 kernel, v7.

Per-core (one batch sample): K = softmax(guided, 0); mask = sign(sparse);
x_{t+1} = mask*x0 + (1-mask) * sum_k K_k * shift_k(x_t), 16 steps.

Layout: xa [120 part, 6 row-slots, 642 cols] fp16, double-buffered.
Row slots: 0 = top halo, 1-4 = rows 4p..4p+3, 5 = bottom halo; col 0/641 pad.

v7 structure (trace-driven redesign of v3):
  - All 9 taps emitted as HALF products (output rows 0:2 = A banks,
    2:4 = B banks).  Order closes the B psum banks ~4us before the DVE
    stream ends, so the B drains + top-halo DMA overlap the remaining
    A-half products and the next iteration's first B-half products start
    with zero DVE stall.
  - z tiles double-buffered by iteration parity: a product of iteration
    t never WAR-stalls on iteration t-1's PE pass reading the same z.
  - Taps 4 (both halves) and 8 (A half) run on the otherwise-idle Pool
    engine (gpsimd tensor_tensor).
  - Halo rows move via SBUF->SBUF DMA (sync + gpsimd queues) instead of
    PE shift-matmuls + ACT drains.
  - m0 (mask*x0) is written into PSUM by per-row ACT copies, each woven
    directly after that row's drain; all z matmuls run start=False.
    PE work is the 9 tap passes only (~9.7us at full clock).
  - Per-row-slot drains (4 ACT copies of 640), order rows 3,2 then 1,0.
  - Two keep-warm dummy matmuls into a scratch psum bank at the
    iteration boundary hold the PE p-state up.
"""

import sys

for _p in ("/opt/trn_rl_repo", "/root/.axon_site/_ro/trn_rl_repo"):
    if _p not in sys.path:
        sys.path.insert(0, _p)

import numpy as np

from concourse import bacc, mybir
from concourse import tile
from concourse.bass_utils import run_bass_kernel_spmd


def dedup_ldweights(nc):
    """Drop InstLdweights whose stationary matches the previous one (PE
    weights persist).  An Ldweights carrying sync info becomes a NoOp."""
    for f in nc.m.functions:
        for bb in f.blocks:
            out = []
            seen_key = None
            changed = False
            for ins in bb.instructions:
                if type(ins).__name__ == "InstLdweights":
                    key = str(ins.ins[0])
                    if key == seen_key:
                        si = ins.sync_info
                        if si is not None and (si.on_wait or si.on_update):
                            out.append(
                                mybir.InstNoOp(
                                    name=ins.name + "-ldwn",
                                    engine=ins.engine,
                                    sync_info=si,
                                )
                            )
                        changed = True
                        continue
                    seen_key = key
                out.append(ins)
            if changed:
                bb.instructions[:] = out


B = 8
H, W = 480, 640
P = 120
RPP = 4
WP = W + 2
NJ = RPP + 2
FLAT = RPP * W       # 2560
HFLAT = FLAT // 2    # 1280
PROP_TIME = 16

FP32 = mybir.dt.float32
FP16 = mybir.dt.float16

TAPS = [(ki - 1, kj - 1) for ki in range(3) for kj in range(3)]

# psum layout: row r lives at words [1024r, 1024r+640) -- 2 banks per row,
# so start=True (m0 base) chunks never share a bank with another row's
# accumulation (start_tensor_calc resets at bank granularity).
PROW = 1024


def row_chunks(row):
    """[(z-flat range, psum-flat range), ...] for one image row."""
    zb, pb = row * W, row * PROW
    return [
        ((zb, zb + 512), (pb, pb + 512)),
        ((zb + 512, zb + W), (pb + 512, pb + W)),
    ]

# DVE product order.  half=1 -> output rows 2:4 (B banks), half=0 -> 0:2.
DVE_PART1 = [(3, 1), (5, 1), (4, 1), (1, 1), (0, 1), (2, 1), (3, 0),
             (6, 1), (8, 1), (7, 1)]
DVE_PART2 = [(5, 0), (4, 0), (1, 0), (0, 0), (2, 0), (6, 0), (8, 0), (7, 0)]


def _rows_view(dram_ap):
    return dram_ap.rearrange("(p r) w -> p r w", p=P)


def build_program(compile_=True):
    nc = bacc.Bacc("TRN2", target_bir_lowering=False, debug=False, num_devices=B)

    guided_d = nc.dram_tensor("guided", [9, H, W], FP32, kind="ExternalInput")
    x_d = nc.dram_tensor("x", [H, W], FP32, kind="ExternalInput")
    sparse_d = nc.dram_tensor("sparse_depth", [H, W], FP32, kind="ExternalInput")
    out_d = nc.dram_tensor("out", [H, W], FP32, kind="ExternalOutput")

    ident_d = nc.inline_tensor(np.eye(P, dtype=np.float16), name="ident_const")

    with tile.TileContext(nc) as tc:
        with (
            tc.tile_pool(name="persist", bufs=1) as persist,
            tc.tile_pool(name="work32", bufs=2) as work32,
            tc.tile_pool(name="psum", bufs=1, space="PSUM") as psump,
        ):
            xa = [
                persist.tile([P, NJ, WP], FP16, tag=f"xa{i}", name=f"xa{i}")
                for i in range(2)
            ]
            wk = [
                persist.tile([P, FLAT], FP16, tag=f"wk{k}", name=f"wk{k}")
                for k in range(9)
            ]
            # z double-buffered by iteration parity
            zz = [
                [
                    persist.tile([P, RPP, W], FP16, tag=f"z{par}_{k}",
                                 name=f"z{par}_{k}")
                    for k in range(9)
                ]
                for par in range(2)
            ]
            m0 = persist.tile([P, FLAT], FP16, tag="m0")
            nomask = persist.tile([P, FLAT], FP16, tag="nomask")
            rf16 = persist.tile([P, FLAT], FP16, tag="rf16")
            ident = persist.tile([P, P], FP16, tag="ident")
            stag = persist.tile([P, RPP, W], FP32, tag="stag")

            psum = psump.tile([P, RPP * PROW], FP32, tag="psum")  # all 8 banks

            # ---- init ----
            nc.vector.memset(xa[0][:], 0.0)
            nc.gpsimd.memset(xa[1][:], 0.0)
            nc.sync.dma_start(out=ident[:], in_=ident_d[:])

            # ---- x load (cast fp32->fp16 via SWDGE) + initial halos ----
            xd = _rows_view(x_d[:])
            nc.gpsimd.dma_start(out=xa[0][:, 1 : 1 + RPP, 1 : 1 + W], in_=xd)
            nc.gpsimd.dma_start(
                out=xa[0][1:P, 0:1, 1 : 1 + W], in_=xd[0 : P - 1, 3:4, :]
            )
            nc.gpsimd.dma_start(
                out=xa[0][0 : P - 1, 5:6, 1 : 1 + W], in_=xd[1:P, 0:1, :]
            )

            # ---- masks (sparse on scalar queue, ahead of its guided) ----
            sp = work32.tile([P, RPP, W], FP32, tag="g32", name="sp32")
            nc.scalar.dma_start(out=sp[:], in_=_rows_view(sparse_d[:]))
            nc.vector.tensor_scalar(
                out=nomask[:], in0=sp.rearrange("p a b -> p (a b)")[:],
                scalar1=0.0, scalar2=None, op0=mybir.AluOpType.is_equal,
            )
            xv = xa[0][:, 1 : 1 + RPP, 1 : 1 + W]
            m0v = m0.rearrange("p (a b) -> p a b", a=RPP)
            nc.vector.tensor_tensor(
                out=m0v[:], in0=nomask.rearrange("p (a b) -> p a b", a=RPP)[:],
                in1=xv, op=mybir.AluOpType.mult,
            )
            nc.vector.tensor_tensor(
                out=m0v[:], in0=xv, in1=m0v[:], op=mybir.AluOpType.subtract
            )

            # ---- guided loads (3-engine spread) + exp; iteration 0 runs on
            # UNNORMALIZED weights, chasing each channel's DMA inside the
            # load wall: z_k = exp(g_k) * tap_k(x0) accumulates on PE while
            # the denominator accumulates on DVE.  Normalization is applied
            # once at drain time: x1 = m0 + (nomask/den) * psum. ----
            dma_engines = [nc.sync, nc.gpsimd, nc.scalar]
            z0 = zz[0]
            wkv0 = [wk[k].rearrange("p (a b) -> p a b", a=RPP) for k in range(9)]
            den16 = persist.tile([P, FLAT], FP16, tag="den16")
            for k in range(9):
                g32 = work32.tile([P, RPP, W], FP32, tag="g32", name=f"g32_{k}")
                dma_engines[k % 3].dma_start(out=g32[:], in_=_rows_view(guided_d[k]))
                nc.scalar.activation(
                    out=wk[k][:], in_=g32.rearrange("p a b -> p (a b)")[:],
                    func=mybir.ActivationFunctionType.Exp,
                )
                dh, dw = TAPS[k]
                nc.vector.tensor_tensor(
                    out=z0[k][:], in0=wkv0[k][:],
                    in1=xa[0][:, 1 + dh : 1 + dh + RPP, 1 + dw : 1 + dw + W],
                    op=mybir.AluOpType.mult,
                )
                if k == 0:
                    nc.vector.tensor_copy(out=den16[:], in_=wk[0][:])
                else:
                    nc.vector.tensor_tensor(
                        out=den16[:], in0=den16[:], in1=wk[k][:],
                        op=mybir.AluOpType.add,
                    )
                zk = z0[k].rearrange("p a b -> p (a b)")
                for row in range(RPP):
                    for (zlo, zhi), (plo, phi) in row_chunks(row):
                        nc.tensor.matmul(
                            out=psum[:, plo:phi],
                            lhsT=ident[:],
                            rhs=zk[:, zlo:zhi],
                            start=(k == 0),
                            stop=(k == 8),
                        )

            # ---- normalization: rf16 = nomask / sum_k exp ----
            den32 = work32.tile([P, RPP, W], FP32, tag="g32", name="den32")
            den32f = den32.rearrange("p a b -> p (a b)")
            nc.vector.tensor_copy(out=den32f[:], in_=den16[:])
            r32 = work32.tile([P, RPP, W], FP32, tag="g32", name="r32")
            r32f = r32.rearrange("p a b -> p (a b)")
            nc.vector.reciprocal_approx_fast(out=r32f[:], in_=den32f[:])
            nc.vector.tensor_tensor(
                out=rf16[:], in0=r32f[:], in1=nomask[:], op=mybir.AluOpType.mult
            )

            # helpers -------------------------------------------------------
            def tap_view(xsrc, k, half):
                dh, dw = TAPS[k]
                r0 = 1 + dh + 2 * half
                c0 = 1 + dw
                return xsrc[:, r0 : r0 + 2, c0 : c0 + W]

            def prod(eng, par, xsrc, k, half):
                wv = wk[k].rearrange("p (a b) -> p a b", a=RPP)
                eng.tensor_tensor(
                    out=zz[par][k][:, 2 * half : 2 * half + 2],
                    in0=wv[:, 2 * half : 2 * half + 2],
                    in1=tap_view(xsrc, k, half),
                    op=mybir.AluOpType.mult,
                )

            def scale_wk(eng, k, half=None):
                lo = 0 if half in (None, 0) else HFLAT
                hi = FLAT if half in (None, 1) else HFLAT
                eng.tensor_tensor(
                    out=wk[k][:, lo:hi], in0=wk[k][:, lo:hi],
                    in1=rf16[:, lo:hi], op=mybir.AluOpType.mult,
                )

            def pe_pass(par, k, half, stop=False):
                zf = zz[par][k].rearrange("p a b -> p (a b)")
                for row in ((2, 3) if half else (0, 1)):
                    for (zlo, zhi), (plo, phi) in row_chunks(row):
                        nc.tensor.matmul(
                            out=psum[:, plo:phi], lhsT=ident[:],
                            rhs=zf[:, zlo:zhi], start=False, stop=stop,
                        )

            def m0_pass(half):
                # m0 base -> psum rows of that half via identity matmul,
                # start=True (each row owns its 2 banks, so the bank reset
                # is safe); PE-internal, so no cross-engine psum edge.
                for row in ((2, 3) if half else (0, 1)):
                    for (zlo, zhi), (plo, phi) in row_chunks(row):
                        nc.tensor.matmul(
                            out=psum[:, plo:phi], lhsT=ident[:],
                            rhs=m0[:, zlo:zhi], start=True, stop=False,
                        )

            def pe_dummy(n):
                # keep-warm matmuls into the row-0 pad words (never read,
                # start=False so no bank reset touches live data).
                for _ in range(n):
                    nc.tensor.matmul(
                        out=psum[:, W : PROW], lhsT=ident[:],
                        rhs=m0[:, 0 : PROW - W], start=False, stop=False,
                    )

            pv = psum.rearrange("p (a b) -> p a b", a=RPP, b=PROW)
            rfv = rf16.rearrange("p (a b) -> p a b", a=RPP)

            def prow(r):
                # psum image row r (640 live words of the 1024-word row)
                return pv[:, r : r + 1, 0:W]

            def drain_slot(r, xaw):
                # psum row r -> xaw slot 1+r (fp32 -> fp16)
                nc.scalar.copy(
                    out=xaw[:, 1 + r : 2 + r, 1 : 1 + W], in_=prow(r)
                )

            def drain_slot_t0(r, xaw):
                # iteration 0 normalizes at drain: x1 = m0 + rf16 * psum_raw
                nc.vector.tensor_tensor(
                    out=stag[:, r : r + 1], in0=rfv[:, r : r + 1],
                    in1=prow(r), op=mybir.AluOpType.mult,
                )
                nc.vector.tensor_tensor(
                    out=xaw[:, 1 + r : 2 + r, 1 : 1 + W], in0=stag[:, r : r + 1],
                    in1=m0v[:, r : r + 1], op=mybir.AluOpType.add,
                )

            def halo_top(xaw):
                # slot0[p] <- slot4[p-1]  (row 4p-1), after slot4 drain
                nc.sync.dma_start(
                    out=xaw[1:P, 0:1, 1 : 1 + W], in_=xaw[0 : P - 1, 4:5, 1 : 1 + W]
                )

            def halo_bot(xaw):
                # slot5[p] <- slot1[p+1]  (row 4p+4), after slot1 drain
                nc.gpsimd.dma_start(
                    out=xaw[0 : P - 1, 5:6, 1 : 1 + W], in_=xaw[1:P, 1:2, 1 : 1 + W]
                )

            odv = _rows_view(out_d[:])

            # ---- 16 propagation iterations (iteration 0 = the raw pass
            # emitted above; only its drain + tail are emitted here) ----
            scaled = set()

            def maybe_scale(eng, k):
                if k not in scaled:
                    scaled.add(k)
                    scale_wk(eng, k)

            for t in range(PROP_TIME):
                par = t % 2
                xar = xa[t % 2]
                xaw = xa[1 - t % 2]
                last = t == PROP_TIME - 1
                first = t == 1

                if t > 0:
                    # part 1: B halves (+ one A filler), closing B banks.
                    # All products on DVE -- Pool tensor_tensor contends with
                    # DVE for SBUF bandwidth (measured 4x DVE slowdown).
                    for k, half in DVE_PART1:
                        if first:
                            maybe_scale(nc.vector, k)
                        prod(nc.vector, par, xar, k, half)
                        if (k, half) == (1, 1):
                            pe_pass(par, 3, 1)
                            pe_pass(par, 5, 1)
                        elif (k, half) == (0, 1):
                            pe_pass(par, 4, 1)
                            pe_pass(par, 1, 1)
                        elif (k, half) == (2, 1):
                            pe_pass(par, 0, 1)
                        elif (k, half) == (3, 0):
                            pe_pass(par, 2, 1)
                            m0_pass(0)      # A-bank base for this iteration
                        elif (k, half) == (6, 1):
                            pe_pass(par, 3, 0)
                            pe_pass(par, 6, 1)
                        elif (k, half) == (8, 1):
                            pe_pass(par, 8, 1)
                        elif (k, half) == (7, 1):
                            pe_pass(par, 7, 1, stop=True)  # closes B banks

                    # B drains (top halo right after slot4 lands).
                    if not last:
                        drain_slot(3, xaw)
                        halo_top(xaw)
                        drain_slot(2, xaw)
                    else:
                        nc.scalar.copy(out=stag[:, 2:4], in_=pv[:, 2:4, 0:W])
                        nc.sync.dma_start(out=odv[:, 2:4], in_=stag[:, 2:4])

                    # part 2: A halves, closing A banks.
                    for k, half in DVE_PART2:
                        prod(nc.vector, par, xar, k, half)
                        if (k, half) == (4, 0):
                            pe_pass(par, 5, 0)
                        elif (k, half) == (1, 0):
                            pe_pass(par, 4, 0)
                        elif (k, half) == (0, 0):
                            pe_pass(par, 1, 0)
                        elif (k, half) == (2, 0):
                            pe_pass(par, 0, 0)
                        elif (k, half) == (6, 0):
                            pe_pass(par, 2, 0)
                            if not last:
                                m0_pass(1)  # B-bank base for t+1
                        elif (k, half) == (8, 0):
                            pe_pass(par, 6, 0)
                        elif (k, half) == (7, 0):
                            pe_pass(par, 8, 0)
                            pe_pass(par, 7, 0, stop=True)  # closes A banks
                            if not last:
                                pe_dummy(2)

                    if not last:
                        drain_slot(1, xaw)
                        drain_slot(0, xaw)
                        halo_bot(xaw)
                    else:
                        nc.scalar.copy(out=stag[:, 0:2], in_=pv[:, 0:2, 0:W])
                        nc.sync.dma_start(out=odv[:, 0:2], in_=stag[:, 0:2])
                else:
                    # iteration 0 tail: DVE-normalized drains + halos + the
                    # m0 base for iteration 1's B banks (PE).
                    drain_slot_t0(3, xaw)
                    halo_top(xaw)
                    drain_slot_t0(2, xaw)
                    m0_pass(1)
                    drain_slot_t0(1, xaw)
                    drain_slot_t0(0, xaw)
                    halo_bot(xaw)

    dedup_ldweights(nc)
    if compile_:
        nc.compile()
    return nc


_CACHED_NC = None


def _get_nc():
    global _CACHED_NC
    if _CACHED_NC is None:
        _CACHED_NC = build_program()
    return _CACHED_NC


def kernel(guided, x, sparse_depth, _trace=False, _trace_kwargs=None):
    guided = np.ascontiguousarray(guided, dtype=np.float32)
    x = np.ascontiguousarray(x, dtype=np.float32)
    sparse_depth = np.ascontiguousarray(sparse_depth, dtype=np.float32)
    assert guided.shape == (B, 9, H, W)

    nc = _get_nc()
    in_maps = [
        {
            "guided": guided[b],
            "x": x[b, 0],
            "sparse_depth": sparse_depth[b, 0],
        }
        for b in range(B)
    ]
    res = run_bass_kernel_spmd(
        nc, in_maps, list(range(B)), trace=_trace, **(_trace_kwargs or {})
    )
    out = np.stack([res.results[b]["out"] for b in range(B)])[:, None]
    if _trace:
        return out.astype(np.float32), res
    return out.astype(np.float32)
